# revision 3
# baseline (speedup 1.0000x reference)
"""MoE (top-2 of 8 experts, relu^2 MLP) — expert-parallel across 8 TRN2 NeuronCores.

Strategy
--------
Host (dispatch only): compute router logits with jax-on-CPU (bit-matching the
reference's arithmetic), take top-2 expert ids per token, and build per-expert
token index lists. Tokens routed to expert e are gathered and transposed into a
[C, cap] buffer for core e (the "all-to-all dispatch by top-k expert id" of the
sharding hint, done at input-staging time).

Device (all the math): each core holds one expert's w1/w2 plus the replicated
router weight. It recomputes the router logits for its gathered tokens
(token-major via PE), finds top-2 with the DVE max8 instruction, forms the
2-way-softmax probability of ITS expert, runs the relu^2 MLP with fp32r
matmuls (full-rate fp32), scales by the probability, and writes yT [C, cap].

Host (unshard): out[token] = sum of the <=2 expert contributions (scatter-add
of the per-core shards).
"""
import os
import sys
import types
import contextlib
import ctypes

import numpy as np

# ---- problem constants (hardcoded per contract) ----
B, T, C, E, D = 4, 2048, 1024, 8, 1024
TOP_K = 2
N = B * T
P = 128
N_CORES = 8
KC = C // P   # contraction chunks, layer 1 (and output chunks, layer 2)
KD = D // P
CHUNK = 512   # token chunk (moving dim; fp32 moving max = 512, >=256 for f32r fast path)

_nc_cache: dict = {}


def _build_program(cap: int):
    """Build + compile the SPMD Bass program for per-core capacity `cap`."""
    import concourse.tile as tile
    from concourse import bacc, mybir
    from concourse.masks import make_identity

    f32 = mybir.dt.float32
    f32r = mybir.dt.float32r
    AF = mybir.ActivationFunctionType
    AX = mybir.AxisListType

    nchunks = cap // CHUNK
    TPC = CHUNK // P  # token tiles per chunk (4)

    nc = bacc.Bacc("TRN2", target_bir_lowering=False, debug=False, num_devices=N_CORES)
    xg_d = nc.dram_tensor("xg", [C, cap], f32, kind="ExternalInput").ap()
    wr_d = nc.dram_tensor("wr", [C, E], f32, kind="ExternalInput").ap()
    w1_d = nc.dram_tensor("w1", [C, D], f32, kind="ExternalInput").ap()
    w2_d = nc.dram_tensor("w2", [D, C], f32, kind="ExternalInput").ap()
    sel_d = nc.dram_tensor("sel", [1, E], f32, kind="ExternalInput").ap()
    yt_d = nc.dram_tensor("yt", [C, cap], f32, kind="ExternalOutput").ap()

    with tile.TileContext(nc) as tc:
        with (
            tc.tile_pool(name="const", bufs=1) as const,
            tc.tile_pool(name="wpool", bufs=1) as wpool,
            tc.tile_pool(name="xpool", bufs=1) as xpool,
            tc.tile_pool(name="hpool", bufs=8) as hpool,
            tc.tile_pool(name="hspool", bufs=3) as hspool,
            tc.tile_pool(name="ypool", bufs=3) as ypool,
            tc.tile_pool(name="ppool", bufs=2) as ppool,
            tc.tile_pool(name="spool", bufs=3) as spool,
            tc.tile_pool(name="psum", bufs=5, space="PSUM") as psum,
            tc.tile_pool(name="pdram", bufs=3, space="DRAM") as pdram,
        ):
            ident = const.tile([P, P], f32, tag="ident")
            make_identity(nc, ident)
            sel_b = const.tile([P, E], f32, tag="selb")
            nc.sync.dma_start(sel_b[:], sel_d.to_broadcast((P, E)))

            # router weight -> f32r tiles [128, 8] per contraction chunk
            wr_sb = [const.tile([P, E], f32r, name=f"wr{k}", tag=f"wr{k}") for k in range(KC)]
            for k in range(KC):
                stgr = spool.tile([P, E], f32, name=f"stgr{k}", tag="stgr")
                nc.sync.dma_start(stgr[:], wr_d[k * P:(k + 1) * P, :])
                nc.vector.tensor_copy(wr_sb[k][:], stgr[:])

            # expert weights / tokens -> f32r tiles via f32 staging + DVE cast
            # (a DMA may not be the producer of an fp32r matmul input; the
            # cast copy performs the required rounding)
            w1_sb = [wpool.tile([P, D], f32r, name=f"w1_{k}", tag=f"w1_{k}") for k in range(KC)]
            w2_sb = [wpool.tile([P, C], f32r, name=f"w2_{k}", tag=f"w2_{k}") for k in range(KD)]
            x_sb = [xpool.tile([P, cap], f32r, name=f"x{k}", tag=f"x{k}") for k in range(KC)]
            halfcap = cap // 2
            with tc.tile_pool(name="stgpool", bufs=2) as stgpool:
                for k in range(KC):
                    for hh in range(2):
                        hsl = slice(hh * halfcap, (hh + 1) * halfcap)
                        stgx = stgpool.tile([P, halfcap], f32, name=f"stgx{k}_{hh}", tag="stg")
                        nc.sync.dma_start(stgx[:], xg_d[k * P:(k + 1) * P, hsl])
                        nc.vector.tensor_copy(x_sb[k][:, hsl], stgx[:])
                for k in range(KC):
                    stg1 = stgpool.tile([P, D], f32, name=f"stg1{k}", tag="stg")
                    nc.sync.dma_start(stg1[:], w1_d[k * P:(k + 1) * P, :])
                    nc.vector.tensor_copy(w1_sb[k][:], stg1[:])
                for k in range(KD):
                    stg2 = stgpool.tile([P, C], f32, name=f"stg2{k}", tag="stg")
                    nc.sync.dma_start(stg2[:], w2_d[k * P:(k + 1) * P, :])
                    nc.vector.tensor_copy(w2_sb[k][:], stg2[:])

            for t in range(nchunks):
                csl = slice(t * CHUNK, (t + 1) * CHUNK)

                # ---- router probs for this chunk (token-major) ----
                vmax = spool.tile([P, TPC, E], f32, tag="vmax")
                ttel = spool.tile([P, TPC, E], f32, tag="ttel")
                le = spool.tile([P, TPC], f32, tag="le")
                for tt in range(TPC):
                    tok = slice(t * CHUNK + tt * P, t * CHUNK + (tt + 1) * P)
                    lp = psum.tile([P, E], f32, tag="lp", bufs=2)
                    for k in range(KC):
                        nc.tensor.matmul(
                            lp[:],
                            x_sb[k][:, tok].bitcast(f32r),
                            wr_sb[k][:],
                            start=(k == 0),
                            stop=(k == KC - 1),
                        )
                    ltok = spool.tile([P, E], f32, name="ltok", tag="ltok")
                    nc.scalar.copy(ltok[:], lp[:])
                    nc.vector.max(vmax[:, tt, :], ltok[:])
                    nc.vector.tensor_mul(ttel[:, tt, :], ltok[:], sel_b[:])
                    nc.vector.reduce_sum(out=le[:, tt:tt + 1], in_=ttel[:, tt, :], axis=AX.X)
                # p = exp(le - v1) / (1 + exp(v2 - v1))   [P, TPC]
                a_t = spool.tile([P, TPC], f32, tag="a")
                nc.vector.tensor_sub(a_t[:], le[:], vmax[:, :, 0])
                b_t = spool.tile([P, TPC], f32, tag="b")
                nc.vector.tensor_sub(b_t[:], vmax[:, :, 1], vmax[:, :, 0])
                ea = spool.tile([P, TPC], f32, tag="ea")
                nc.scalar.activation(ea[:], a_t[:], AF.Exp)
                eb = spool.tile([P, TPC], f32, tag="eb")
                nc.scalar.activation(eb[:], b_t[:], AF.Exp)
                nc.vector.tensor_scalar_add(eb[:], eb[:], 1.0)
                rec = spool.tile([P, TPC], f32, tag="rec")
                nc.vector.reciprocal(rec[:], eb[:])
                p_all = spool.tile([P, TPC], f32, tag="pall")
                nc.vector.tensor_mul(p_all[:], ea[:], rec[:])
                # token-major [P, TPC] -> row [1, CHUNK] -> broadcast [P, CHUNK]
                ptp = psum.tile([TPC, P], f32, tag="ptp", bufs=1)
                nc.tensor.transpose(ptp[:], p_all[:], ident[:])
                pts = spool.tile([TPC, P], f32, tag="pts")
                nc.scalar.copy(pts[:], ptp[:])
                pdr = pdram.tile([TPC, P], f32, tag="pdr")
                nc.sync.dma_start(pdr[:], pts[:])
                p_b = ppool.tile([P, CHUNK], f32, tag="pb")
                nc.sync.dma_start(
                    p_b[:],
                    pdr[:].rearrange("a b -> (a b)").unsqueeze(0).to_broadcast((P, CHUNK)),
                )

                # ---- layer 1: hT[m] = relu(w1[:,m].T @ xT)^2 ----
                h_tiles = []
                for m in range(KD):
                    hp = psum.tile([P, CHUNK], f32, tag="psA", bufs=5)
                    for k in range(KC):
                        nc.tensor.matmul(
                            hp[:],
                            w1_sb[k][:, m * P:(m + 1) * P].bitcast(f32r),
                            x_sb[k][:, csl].bitcast(f32r),
                            start=(k == 0),
                            stop=(k == KC - 1),
                        )
                    hs = hspool.tile([P, CHUNK], f32, name=f"hs{m}", tag="hs")
                    nc.scalar.activation(hs[:], hp[:], AF.Relu)
                    h2 = hpool.tile([P, CHUNK], f32r, name=f"h{m}", tag="h")
                    nc.vector.tensor_mul(h2[:], hs[:], hs[:])
                    h_tiles.append(h2)

                # ---- layer 2: yT[c] = (w2[:,c].T @ hT) * p ----
                for c8 in range(KC):
                    yp = psum.tile([P, CHUNK], f32, tag="psA", bufs=5)
                    for k in range(KD):
                        nc.tensor.matmul(
                            yp[:],
                            w2_sb[k][:, c8 * P:(c8 + 1) * P].bitcast(f32r),
                            h_tiles[k][:].bitcast(f32r),
                            start=(k == 0),
                            stop=(k == KD - 1),
                        )
                    ys = ypool.tile([P, CHUNK], f32, name=f"y{c8}", tag="y")
                    nc.vector.tensor_mul(ys[:], yp[:], p_b[:])
                    nc.sync.dma_start(yt_d[c8 * P:(c8 + 1) * P, csl], ys[:])

    nc.compile()
    return nc


# ---------------------------------------------------------------------------
# host-side dispatch + unshard
# ---------------------------------------------------------------------------

def _route_cpu(xf: np.ndarray, w_router: np.ndarray):
    """Router top-2 on CPU via jax (matches the reference's arithmetic)."""
    import jax

    cpu = jax.devices("cpu")[0]
    with jax.default_device(cpu):
        import jax.numpy as jnp

        logits = jnp.asarray(xf) @ jnp.asarray(w_router)
        top_vals, top_idx = jax.lax.top_k(logits, TOP_K)
        return np.asarray(top_idx)


def _profile_hook_ctx():
    """Register the axon NTFF profile hook (timing runs only)."""
    so_path = "/opt/axon/libaxon_pjrt.so"
    lib = ctypes.CDLL(so_path)
    if not hasattr(lib, "axon_start_nrt_profile"):
        return False
    lib.axon_start_nrt_profile.argtypes = [ctypes.POINTER(ctypes.c_int64), ctypes.c_size_t]
    lib.axon_start_nrt_profile.restype = ctypes.c_int64
    lib.axon_stop_nrt_profile.argtypes = [ctypes.c_char_p]
    lib.axon_stop_nrt_profile.restype = ctypes.c_int64

    @contextlib.contextmanager
    def _hook(output_dir, device_ids):
        import jax

        jax.devices()
        if device_ids:
            ids = (ctypes.c_int64 * len(device_ids))(*device_ids)
            rc = lib.axon_start_nrt_profile(ids, len(device_ids))
        else:
            rc = lib.axon_start_nrt_profile(None, 0)
        if rc != 0:
            raise RuntimeError(f"axon_start_nrt_profile rc={rc}")
        try:
            yield
        finally:
            n = lib.axon_stop_nrt_profile(str(output_dir).encode())
            print(f"profile: {n} file(s) written to {output_dir}")

    mod = types.ModuleType("antenv.axon_hooks")
    mod.get_axon_ntff_profile_hook = lambda: _hook
    sys.modules["antenv.axon_hooks"] = mod
    # the trace path uploads artifacts to a bucket; keep it local
    import concourse.bass_utils as bu

    bu.upload_artifacts = lambda tmpdir: tmpdir
    return True


_LAST_EXEC_NS = None  # set when MOE_KERNEL_TRACE=1


def kernel(x, w_router, w1, w2):
    global _LAST_EXEC_NS
    from concourse.bass_utils import run_bass_kernel_spmd

    x = np.asarray(x, dtype=np.float32)
    w_router = np.asarray(w_router, dtype=np.float32)
    w1 = np.asarray(w1, dtype=np.float32)
    w2 = np.asarray(w2, dtype=np.float32)

    xf = np.ascontiguousarray(x.reshape(N, C))
    top_idx = _route_cpu(xf, w_router)  # [N, 2]

    idx_lists = []
    max_cnt = 1
    for e in range(E):
        idx_e = np.nonzero((top_idx[:, 0] == e) | (top_idx[:, 1] == e))[0]
        idx_lists.append(idx_e)
        max_cnt = max(max_cnt, len(idx_e))
    cap = ((max_cnt + CHUNK - 1) // CHUNK) * CHUNK

    if cap not in _nc_cache:
        _nc_cache[cap] = _build_program(cap)
    nc = _nc_cache[cap]

    in_maps = []
    for e in range(E):
        idx_e = idx_lists[e]
        xg = np.zeros((C, cap), dtype=np.float32)
        if len(idx_e):
            xg[:, :len(idx_e)] = xf[idx_e].T
        sel = np.zeros((1, E), dtype=np.float32)
        sel[0, e] = 1.0
        in_maps.append({
            "xg": xg,
            "wr": w_router,
            "w1": np.ascontiguousarray(w1[e]),
            "w2": np.ascontiguousarray(w2[e]),
            "sel": sel,
        })

    trace = os.environ.get("MOE_KERNEL_TRACE", "0") == "1"
    kwargs = {}
    if trace and _profile_hook_ctx():
        kwargs = dict(trace=True)
        if os.environ.get("MOE_TRACE_ALL_CORES", "0") == "1":
            kwargs["trace_cores"] = list(range(N_CORES))

    res = run_bass_kernel_spmd(nc, in_maps, core_ids=list(range(N_CORES)), **kwargs)
    _LAST_EXEC_NS = res.exec_time_ns

    out = np.zeros((N, C), dtype=np.float32)
    for e in range(E):
        idx_e = idx_lists[e]
        if len(idx_e):
            out[idx_e] += res.results[e]["yt"][:, :len(idx_e)].T
    return out.reshape(B, T, C)
